# revision 61
# baseline (speedup 1.0000x reference)
"""Trainium2 Bass kernel for nn_MultiHeadedAttention_33835752358170.

Shapes (hardcoded): x [4, 2048, 1024] f32, w_in [192, 1024], b_in [192],
w_out [1024, 64], b_out [1024].  Module quirk: d_k = 64 total across 16
heads -> head_dim = 4.  Scale 1/sqrt(64) = 1/8 folded into q weights.

Sharding: 8 cores = 4 batches x 2 query-halves.  Each core: K/V over the
full S=2048, attention + output projection for its own 1024 query rows.
Per-core x is column-rotated so the core's own query half occupies token
columns 0:1024 -- the program always consumes "my queries first", and
attention is permutation-invariant over keys.

v5 design:
- masked-q scores: one K=64 matmul pair per (chunk, head) with compact k
  as the stationary operand and a per-head masked copy of compact q as
  the moving operand (qmask rows 64:128, slot h; zeros outside head h's
  4 rows).  No strip layouts, placement matmuls, or strip copies.
- qmask built by 16 same-partition strided DMAs (+64 partition shift)
  from the packed q/k projection; zero gaps memset on the idle GPSIMD.
- single pass: outs [128, 8, 128] f32 accumulates all 1024 queries; exp
  split ACT (exact Exp, [128,1024] tiles x2 bufs) / DVE (Schraudolph
  bit-trick, [128,512] tiles x2 bufs) in separate PSUM rings so the
  rings never cross-couple; AV matmuls lag LAG units behind scores.
- kc-sh1 projection interleaved at unit BOUNDARY_K, v-sh1 chunks at the
  c=8 boundary, borrowing mainloop PSUM ring slots.
- fp16 output tensor (halves the store DMA); host upcasts.
"""

import math

import numpy as np
import ml_dtypes

import concourse.bass as bass
import concourse.mybir as mybir
import concourse.tile as tile
from concourse import bacc
from concourse.bass_utils import run_bass_kernel_spmd

BF16 = ml_dtypes.bfloat16
F16 = np.float16
F32 = np.float32

B, S, DM = 4, 2048, 1024
NH, DK = 16, 64
HD = 4          # head dim
SQ = 1024       # query rows per core
NC_CORES = 8

LOG2E = 1.4426950408889634
EXP_A = 128.0 * LOG2E
EXP_B = 128.0 * 127.0

# mainloop units: key chunk c, head h; all 1024 queries per unit
UNITS = [(c, h) for c in range(16) for h in range(NH)]
N_UNITS = len(UNITS)              # 256
ACT_UNITS = 143                   # exp units on ScalarE (rest DVE)
LAG = 4                           # AV trails scores by LAG units
BOUNDARY_K = 122                  # unit where the kc-sh1 projection runs
BOUNDARY = 128                    # unit where v-sh1 chunks start

_cache = {}
_DEBUG = False


def _unit_on_act(u):
    return (u * ACT_UNITS) // N_UNITS != ((u + 1) * ACT_UNITS) // N_UNITS


def _build_nc():
    f32 = mybir.dt.float32
    f16 = mybir.dt.float16
    bf16 = mybir.dt.bfloat16
    i16 = mybir.dt.int16
    Exp = mybir.ActivationFunctionType.Exp
    MUL = mybir.AluOpType.mult
    ADD = mybir.AluOpType.add

    nc = bacc.Bacc("TRN2", target_bir_lowering=False, debug=False)

    # ---- DRAM I/O ----
    d_xT = nc.dram_tensor("xT", [DM, S], bf16, kind="ExternalInput").ap()
    # weight layout pre-arranged on host to the SBUF [128, k, n] shape so
    # the DMA moves full-rate runs; qk cols 0:128, v cols 128:192
    d_wqkv = nc.dram_tensor("wqkv", [128, 8, 192], bf16,
                            kind="ExternalInput").ap()
    d_bqk = nc.dram_tensor("bqk", [128, 1], f32, kind="ExternalInput").ap()
    d_wo = nc.dram_tensor("wo", [DK + 1, DM], bf16, kind="ExternalInput").ap()
    d_id = nc.dram_tensor("ident", [128, 128], bf16, kind="ExternalInput").ap()
    d_y = nc.dram_tensor("y", [SQ, DM], f16, kind="ExternalOutput").ap()
    if _DEBUG:
        d_dbg_qk = nc.dram_tensor("dbg_qk", [128, SQ], bf16,
                                  kind="ExternalOutput").ap()
        d_dbg_qm = nc.dram_tensor("dbg_qm", [128, NH, SQ], bf16,
                                  kind="ExternalOutput").ap()
        d_dbg_va = nc.dram_tensor("dbg_va", [128, 16, 128], bf16,
                                  kind="ExternalOutput").ap()
        d_dbg_nrm = nc.dram_tensor("dbg_nrm", [128, 8, 72], bf16,
                                   kind="ExternalOutput").ap()

    with tile.TileContext(nc) as tc:
        with tc.tile_pool(name="const", bufs=1) as cp:
            # ---- SBUF residents ----
            wqkv_sb = cp.tile([128, 8, 192], bf16)
            wqk_sb = wqkv_sb[:, :, 0:128]
            wv_sb = wqkv_sb[:, :, 128:192]
            bqk_sb = cp.tile([128, 1], f32)
            wo_sb = cp.tile([DK + 1, DM], bf16)
            id_sb = cp.tile([128, 128], bf16)
            xT_sb = cp.tile([128, 8, S], bf16)
            qk_sb = cp.tile([128, SQ], bf16)       # rows 0:64 qc, 64:128 kc0
            kc1_sb = cp.tile([128, SQ], bf16)      # kc, token cols 1024:2048
            qmask = cp.tile([128, NH, SQ], bf16)   # rows 64:128: masked q
            va = cp.tile([128, 16, 128], bf16)     # per chunk c: 8h+x cols
            nrm = cp.tile([128, 8, 72], bf16)      # normalized out + ones col
            zlhs = cp.tile([128, 128], bf16)

            nc.sync.dma_start(out=wqkv_sb[:, :, 0:128],
                              in_=d_wqkv[:, :, 0:128])
            # x chunks: query-half columns (0:1024) first, 2 kc per DMA
            xq_v = d_xT[0:1024, 0:1024].rearrange("(k p) n -> p k n", p=128)
            for kc2 in range(4):
                nc.sync.dma_start(
                    out=xT_sb[:, 2 * kc2:2 * kc2 + 2, 0:1024],
                    in_=xq_v[:, 2 * kc2:2 * kc2 + 2, :])
            nc.sync.dma_start(out=bqk_sb, in_=d_bqk)
            nc.sync.dma_start(out=wqkv_sb[:, :, 128:192],
                              in_=d_wqkv[:, :, 128:192])

            nc.vector.memset(zlhs, 0.0)
            nc.scalar.activation(nrm[0:1, 0, 0:1], zlhs[0:1, 0:1], Exp)
            nc.vector.memset(va, 0.0)
            va_h = va.rearrange("p c (h x) -> p c h x", x=8)
            nc.vector.memset(va_h[:, :, :, 4:5], 1.0)
            nc.vector.memset(nrm[:, :, DK:DK + 1], 1.0)
            # qmask zero gaps: h=0 DVE, h=1 ACT, rest on the idle GPSIMD
            nc.vector.memset(qmask[64:128, 0, :], 0.0)
            nc.vector.memset(qmask[64:128, 1, :], 0.0)
            for h in range(2, NH):
                nc.gpsimd.memset(qmask[64:128, h, :], 0.0)

            # ---- phase 1: packed q+k projection ----
            with tc.tile_pool(name="pp", bufs=1, space="PSUM") as pp:
                qk_ps = pp.tile([128, SQ], f32, tag="p")
                # PE p-state warm-up during the input DMA wait; lands in
                # qk_ps which the first real matmul (start=True) clears.
                for w in range(24):
                    nc.tensor.matmul(qk_ps[0:64, 0:64], zlhs[:, 0:64],
                                     zlhs[:, 0:64], start=True, stop=True)
                # packed q+k projection over the query-half columns
                for kc in range(8):
                    for nh in range(2):
                        nc.tensor.matmul(
                            qk_ps[:, nh * 512:(nh + 1) * 512],
                            wqk_sb[:, kc, :],
                            xT_sb[:, kc, nh * 512:(nh + 1) * 512],
                            start=(kc == 0), stop=(kc == 7))
                nc.scalar.add(qk_sb[:, 0:512], qk_ps[:, 0:512], bqk_sb)
                nc.vector.tensor_scalar_add(qk_sb[:, 512:1024],
                                            qk_ps[:, 512:1024], bqk_sb)

                # qmask fills: head h=4j+s owns compact rows {16s+4d+j};
                # same strided row pattern shifted +64 partitions
                for h in range(NH):
                    j, s = h // 4, h % 4
                    r0 = 16 * s + j
                    nc.sync.dma_start(
                        out=qmask[64 + r0:64 + r0 + 13:4, h, :],
                        in_=qk_sb[r0:r0 + 13:4, :])

                # second x half + epilogue-only weights
                for kc in range(8):
                    nc.sync.dma_start(
                        out=xT_sb[:, kc, 1024:2048],
                        in_=d_xT[kc * 128:(kc + 1) * 128, 1024:2048])
                nc.sync.dma_start(out=wo_sb, in_=d_wo)
                nc.sync.dma_start(out=id_sb, in_=d_id)

            # ---- phase 2: attention mainloop (single pass) ----
            with tc.tile_pool(name="op", bufs=1, space="PSUM") as op, \
                 tc.tile_pool(name="spa", bufs=2, space="PSUM") as spA, \
                 tc.tile_pool(name="spd", bufs=2, space="PSUM") as spD, \
                 tc.tile_pool(name="ep", bufs=LAG + 4) as ep:
                # attnout accumulator: [q(128), ql(8), 8h+x(128)], x=4 denom
                outs = op.tile([128, 8, 128], f32, tag="outs")
                outs_f = outs.rearrange("p a b -> p (a b)")
                w_f = wqkv_sb.rearrange("p a b -> p (a b)")
                for nh in range(2):
                    nc.tensor.matmul(outs_f[:, nh * 512:(nh + 1) * 512],
                                     zlhs, w_f[:, 0:512],
                                     start=True, stop=False)

                ets = [None] * N_UNITS

                def emit_av(u, last):
                    et = ets[u]
                    c, h = UNITS[u]
                    for ql in range(8):
                        nc.tensor.matmul(
                            outs[:, ql, 8 * h:8 * h + 5],
                            et[:, ql * 128:(ql + 1) * 128],
                            va[:, c, 8 * h:8 * h + 5],
                            start=False, stop=last)
                    ets[u] = None

                def pv_chunk(c2):
                    # v projection chunk; borrows an spA ring slot
                    pv = spA.tile([128, SQ], f32, tag="sa")
                    for kc in range(8):
                        nc.tensor.matmul(
                            pv[:, 0:64],
                            xT_sb[:, kc, c2 * 128:(c2 + 1) * 128],
                            wv_sb[:, kc, :],
                            start=(kc == 0), stop=(kc == 7))
                    pv_v = pv[:, 0:64].rearrange("p (h d) -> p h d", d=4)
                    if c2 % 2 == 0:
                        nc.vector.tensor_copy(va_h[:, c2, :, 0:4], pv_v)
                    else:
                        nc.scalar.copy(va_h[:, c2, :, 0:4], pv_v)

                def kc1_block():
                    # kc over token cols 1024:2048; borrows an spA slot.
                    kc1_ps = spA.tile([128, SQ], f32, tag="sa")
                    for kc in range(8):
                        for nh in range(2):
                            nc.tensor.matmul(
                                kc1_ps[64:128, nh * 512:(nh + 1) * 512],
                                wqk_sb[:, kc, 64:128],
                                xT_sb[:, kc,
                                      1024 + nh * 512:1024 + (nh + 1) * 512],
                                start=(kc == 0), stop=(kc == 7))
                    nc.scalar.add(kc1_sb[64:128, :], kc1_ps[64:128, :],
                                  bqk_sb[64:128, :])

                # v chunks 0..7 (phase-1 data, before the units)
                for c2 in range(8):
                    pv_chunk(c2)

                for u, (c, h) in enumerate(UNITS):
                    if u == BOUNDARY_K:
                        kc1_block()
                    if u == BOUNDARY:
                        for c2 in range(8, 16):
                            pv_chunk(c2)
                    ksrc = qk_sb if c < 8 else kc1_sb
                    klhs = ksrc[64:128, (c % 8) * 128:(c % 8 + 1) * 128]
                    et = ep.tile([128, 1024], bf16, tag="et")
                    if _unit_on_act(u):
                        st = spA.tile([128, 1024], f32, tag="sa")
                        for nh in range(2):
                            nc.tensor.matmul(
                                st[:, nh * 512:(nh + 1) * 512], klhs,
                                qmask[64:128, h, nh * 512:(nh + 1) * 512],
                                start=True, stop=True)
                        nc.scalar.activation(et, st, Exp)
                    else:
                        et_i = et.bitcast(i16)
                        for nh in range(2):
                            std = spD.tile([128, 512], f32, tag="sd")
                            nc.tensor.matmul(
                                std, klhs,
                                qmask[64:128, h, nh * 512:(nh + 1) * 512],
                                start=True, stop=True)
                            nc.vector.tensor_scalar(
                                out=et_i[:, nh * 512:(nh + 1) * 512],
                                in0=std,
                                scalar1=EXP_A, scalar2=EXP_B,
                                op0=MUL, op1=ADD)
                    ets[u] = et
                    if u >= LAG:
                        emit_av(u - LAG, last=False)
                for u in range(N_UNITS - LAG, N_UNITS):
                    emit_av(u, last=(u == N_UNITS - 1))

                # normalize into nrm (still inside op scope: reads outs);
                # one reciprocal + one multiply over all 8 q-chunks
                with tc.tile_pool(name="ns", bufs=1) as nsp:
                    o_all = outs.rearrange("p a (h x) -> p a h x", x=8)
                    rec = nsp.tile([128, 8, NH, 1], f32, tag="rec")
                    nc.vector.reciprocal(rec, o_all[:, :, :, 4:5])
                    nrm_va = nrm[:, :, 0:DK].rearrange(
                        "p a (h d) -> p a h d", d=4)
                    nc.vector.tensor_mul(
                        nrm_va, o_all[:, :, :, 0:4],
                        rec.broadcast_to([128, 8, NH, 4]))

            if _DEBUG:
                nc.sync.dma_start(out=d_dbg_qk, in_=qk_sb)
                nc.sync.dma_start(out=d_dbg_qm, in_=qmask)
                nc.sync.dma_start(out=d_dbg_va, in_=va)
                nc.sync.dma_start(out=d_dbg_nrm, in_=nrm)

            # ---- phase 3: transpose + output projection epilogue ----
            with tc.tile_pool(name="fp", bufs=2, space="PSUM") as fp, \
                 tc.tile_pool(name="tpp", bufs=3, space="PSUM") as tpp, \
                 tc.tile_pool(name="fs", bufs=5) as fs:
                ys2 = None
                for qc in range(8):
                    tp = tpp.tile([DK + 1, 128], bf16, tag="tp")
                    nc.tensor.transpose(tp, nrm[:, qc, 0:DK + 1], id_sb)
                    at = fs.tile([DK + 1, 128], bf16, tag="at")
                    if qc % 2 == 1:
                        nc.scalar.copy(at, tp)
                    else:
                        nc.vector.tensor_copy(at, tp)
                    yp = fp.tile([128, DM], f32, tag="yp")
                    for nd in range(2):
                        nc.tensor.matmul(
                            yp[:, nd * 512:(nd + 1) * 512], at,
                            wo_sb[:, nd * 512:(nd + 1) * 512],
                            start=True, stop=True)
                    if qc < 6:
                        # paired 2-chunk stores
                        if qc % 2 == 0:
                            ys2 = fs.tile([128, 2, DM], f16, tag="ys")
                        half = ys2[:, qc % 2, :]
                        if qc in (1, 3, 5):
                            nc.vector.tensor_copy(half, yp)
                        else:
                            nc.scalar.copy(half, yp)
                        if qc % 2 == 1:
                            dst = d_y[(qc - 1) * 128:(qc + 1) * 128,
                                      :].rearrange("(a p) m -> p a m", a=2)
                            nc.sync.dma_start(out=dst, in_=ys2)
                    else:
                        # last two chunks stream individually
                        ys1 = fs.tile([128, DM], f16, tag="y1")
                        if qc == 6:
                            nc.vector.tensor_copy(ys1, yp)
                        else:
                            nc.scalar.copy(ys1, yp)
                        nc.sync.dma_start(
                            out=d_y[qc * 128:(qc + 1) * 128, :], in_=ys1)

    nc.compile()
    return nc


def _perm():
    # compact row r = 16s + 4d + j holds (head 4j+s, dim d) = w_in row
    # 4*(4j+s)+d
    p = np.zeros(DK, np.int64)
    for s in range(4):
        for d in range(4):
            for j in range(4):
                p[16 * s + 4 * d + j] = 4 * (4 * j + s) + d
    return p


def _prep_consts(w_in, b_in, w_out, b_out):
    w64 = w_in.astype(np.float64)
    perm = _perm()
    wq = (w64[0:64] / 8.0)[perm].T          # [DM, 64]
    wk = w64[64:128][perm].T                # [DM, 64]
    wv = w64[128:192].T                     # [DM, 64], natural d order
    bq = (b_in[0:64].astype(np.float64) / 8.0)[perm]
    bk = b_in[64:128].astype(np.float64)[perm]
    bv = b_in[128:192].astype(np.float64)

    wqk = np.concatenate([wq, wk], axis=1)  # [DM, 128]
    bqk = np.concatenate([bq, bk]).reshape(128, 1)

    wo = np.zeros((DK + 1, DM), np.float64)
    wo[0:DK, :] = w_out.astype(np.float64).T    # row 4h+d = w_out[:, 4h+d]
    wo[DK, :] = b_out.astype(np.float64) + w_out.astype(np.float64) @ bv

    wqkv = np.concatenate([wqk, wv], axis=1)         # [DM, 192]
    return {
        "wqkv": np.ascontiguousarray(
            wqkv.reshape(8, 128, 192).transpose(1, 0, 2)).astype(BF16),
        "bqk": bqk.astype(F32),
        "wo": wo.astype(BF16),
        "ident": np.eye(128, dtype=F32).astype(BF16),
    }


def kernel(x, w_in, b_in, w_out, b_out, _trace=False, **kw):
    x = np.asarray(x, F32)
    consts = _prep_consts(np.asarray(w_in, F32), np.asarray(b_in, F32),
                          np.asarray(w_out, F32), np.asarray(b_out, F32))
    if "nc" not in _cache:
        _cache["nc"] = _build_nc()
    nc = _cache["nc"]

    in_maps = []
    for core in range(NC_CORES):
        b, half = divmod(core, 2)
        xb = x[b].T.astype(BF16)                     # [DM, S]
        if half == 1:
            xb = np.concatenate([xb[:, SQ:], xb[:, :SQ]], axis=1)
        m = dict(consts)
        m["xT"] = np.ascontiguousarray(xb)
        in_maps.append(m)

    res = run_bass_kernel_spmd(nc, in_maps, list(range(NC_CORES)),
                               trace=_trace)
    _cache["res"] = res
    out = np.empty((B, S, DM), F32)
    for core in range(NC_CORES):
        b, half = divmod(core, 2)
        out[b, half * SQ:(half + 1) * SQ, :] = res.results[core]["y"]
    if _trace:
        return out, res
    return out
